# revision 17
# baseline (speedup 1.0000x reference)
"""Graph-transformer layer (GTLayer) on 8 Trainium2 NeuronCores.

Node-parallel sharding with host-side edge binning: nodes are assigned to
8 cores x 49 blocks of 128 by a degree-balanced packing (greedy LPT +
swap refinement), so every block carries ~2041 edges and the per-block
tile count hits its floor t_b = 16; the output permutation is undone on
the host. Each core receives the edges destined to its blocks, padded to
t_b tiles of 128 edge slots.

Per-core device program:
  - Phase A: build a combined table kvNodes[50048, 196xf32] where row r =
    [k(r) 128xf32 | per head: (v(r) 16, 1.0) as 136xbf16], so ONE indirect
    gather per edge tile fetches k and v, and one vector multiply
    produces both the weighted values and the exp-weight column of the
    segment sum. Embeddings arrive host-pre-transposed, so projections
    are straight matmuls (no PE transposes in phase A).
  - Phase B per block: Q_block = embT_local-slice @ qT (PE). Per tile:
    gather kv rows by source node (the only Pool-engine call per tile
    ~1.04us - the structural bottleneck), build the scatter one-hot on
    DVE, transpose it on PE and matmul against Q_block to select
    per-edge q rows (q never touches HBM); whole-block strip ops do
    qk-dot / clip / exp / weighting; one-hot matmuls scatter-add
    [att*v | expw] into PSUM; normalize + residual + LayerNorm, with
    1/sqrt(var+eps) = exp(-0.5*ln(var+eps)) so the Activation engine
    never switches function tables (Ln/Exp/Copy share one set).

f32 is kept on the logit path (tables, q, qk products); bf16 only where
exact (one-hot) or provably small (v weights, exp weights, scatter
matmul operands). Measured rel err ~4e-3 vs the f64 reference.

All cores run one identical program; per-core behavior differs only
through input data (binned index arrays + local embed slices).
"""

import numpy as np
import ml_dtypes

import concourse.bass as bass
import concourse.bacc as bacc
import concourse.tile as tile
from concourse import mybir
from concourse.bass_utils import run_bass_kernel_spmd

N = 50000
E = 800000
D = 128
H = 8
HD = 16
NCORES = 8
NPC = N // NCORES  # 6250 nodes per core
NB = (NPC + 127) // 128  # 49 blocks of 128 nodes per core
NBP = NB * 128  # 6272 padded local nodes
NPAD = ((N + 127) // 128) * 128  # 50048 padded table rows
NT = NPAD // 128  # 391 table blocks
KVW = D + H * (HD + 1) // 2  # 196 f32: [k(128) f32 | (v(16),1)*8 bf16]
XW = H * (HD + 1)  # 136: per-edge [att*v | expw] strip width
SLAB = 8  # phase-A blocks per DMA slab

F32 = mybir.dt.float32
BF16 = mybir.dt.bfloat16
I32 = mybir.dt.int32

BF = ml_dtypes.bfloat16


def _ap(base: bass.AP, off: int, dims) -> bass.AP:
    """Strided view of an SBUF tile AP: keep partition dim, replace free dims."""
    return bass.AP(tensor=base.tensor, offset=base.offset + off, ap=[base.ap[0], *dims])


def build_program(t_b: int, repeat: int = 1) -> bass.Bass:
    nc = bacc.Bacc(None, num_swdge_queues=4)

    embT_glob = nc.dram_tensor("embT_glob", [128, NPAD], F32, kind="ExternalInput")
    embT_local = nc.dram_tensor("embT_local", [128, NBP], F32, kind="ExternalInput")
    emb_local = nc.dram_tensor("emb_local", [NBP, D], F32, kind="ExternalInput")
    qT = nc.dram_tensor("qT", [D, D], F32, kind="ExternalInput")
    kT = nc.dram_tensor("kT", [D, D], F32, kind="ExternalInput")
    vT = nc.dram_tensor("vT", [D, D], F32, kind="ExternalInput")
    lnsc = nc.dram_tensor("lnsc", [D], F32, kind="ExternalInput")
    lnb = nc.dram_tensor("lnb", [D], F32, kind="ExternalInput")
    iota_in = nc.dram_tensor("iota", [128, 128], BF16, kind="ExternalInput")
    ident_in = nc.dram_tensor("ident", [128, 128], BF16, kind="ExternalInput")
    lloc_d = nc.dram_tensor("lloc", [128, NB * t_b], F32, kind="ExternalInput")
    cidx_d = nc.dram_tensor("cidx", [128, NB * t_b], I32, kind="ExternalInput")

    kvNodes = nc.dram_tensor("kvNodes", [NPAD, KVW], F32)

    out_d = nc.dram_tensor("out", [NBP, D], F32, kind="ExternalOutput")

    with tile.TileContext(nc) as tc:
        with tc.tile_pool(name="singles", bufs=1) as singles:
            # ---- one-time constants ----
            iota_bf = singles.tile([128, 128], BF16)
            nc.sync.dma_start(iota_bf[:], iota_in[:])
            ident_bf = singles.tile([128, 128], BF16)
            nc.sync.dma_start(ident_bf[:], ident_in[:])

            lnsc_t = singles.tile([128, 128], F32)
            nc.sync.dma_start(
                out=lnsc_t[:],
                in_=bass.AP(tensor=lnsc, offset=0, ap=[[0, 128], [1, 128]]),
            )
            lnb_t = singles.tile([128, 128], F32)
            nc.sync.dma_start(
                out=lnb_t[:],
                in_=bass.AP(tensor=lnb, offset=0, ap=[[0, 128], [1, 128]]),
            )
            eps_t = singles.tile([128, 1], F32)
            nc.vector.memset(eps_t[:], 1e-6)

            qT_t = singles.tile([128, 128], F32)
            nc.sync.dma_start(qT_t[:], qT[:])
            kT_t = singles.tile([128, 128], F32)
            nc.sync.dma_start(kT_t[:], kT[:])
            vT_t = singles.tile([128, 128], F32)
            nc.sync.dma_start(vT_t[:], vT[:])

            # whole-core preloads: indices + transposed local embeds.
            # cidx gates the very first gather, so it loads up front; the
            # rest load AFTER phase A's table DMAs are issued, keeping the
            # last kvNodes write (which gates the Pool gather stream) off
            # the phase-A critical path.
            cix_all = singles.tile([128, NB * t_b], I32)
            nc.sync.dma_start(cix_all[:], cidx_d[:])
            lloc_all = singles.tile([128, NB * t_b], F32)
            embTl = singles.tile([128, NBP], F32)

            for _rep in range(repeat):
                phase_a(nc, tc, embT_glob, kvNodes, kT_t, vT_t)
                nc.sync.dma_start(lloc_all[:], lloc_d[:])
                nc.sync.dma_start(embTl[:], embT_local[:])
                phase_b(
                    nc, tc, t_b, emb_local, kvNodes, out_d,
                    iota_bf, ident_bf, qT_t, lnsc_t, lnb_t, eps_t,
                    lloc_all, cix_all, embTl,
                )
    nc.finalize()
    return nc


def phase_a(nc, tc, embT_glob, kvNodes, kT_t, vT_t):
    """kvNodes[r] = [k(r)(128) | per head: (v(r)(16), 1.0)] for all nodes."""
    with (
        tc.tile_pool(name="tA", bufs=3) as tA,
        tc.tile_pool(name="tAw", bufs=3) as tAw,
        tc.tile_pool(name="psB", bufs=2, space="PSUM") as psB,
    ):
        for s0 in range(0, NT, SLAB):
            ns = min(SLAB, NT - s0)
            embT_sl = tA.tile([128, ns * 128], F32)
            nc.sync.dma_start(embT_sl[:], embT_glob[:, s0 * 128 : s0 * 128 + ns * 128])
            kvsl = tAw.tile([128, ns * KVW], F32)
            kvsl_bf = kvsl[:].bitcast(BF16)
            # per-head trailing 1.0 columns (bf16 col 256 + h*17 + 16)
            nc.vector.memset(
                _ap(kvsl_bf, 2 * D + HD, [[2 * KVW, ns], [HD + 1, H]]), 1.0
            )
            for j in range(ns):
                lhs = embT_sl[:, j * 128 : (j + 1) * 128]
                kp = psB.tile([128, 128], F32)
                nc.tensor.matmul(kp[:], lhsT=lhs, rhs=kT_t[:], start=True, stop=True)
                nc.scalar.copy(kvsl[:, j * KVW : j * KVW + D], kp[:])

                vp = psB.tile([128, 128], F32)
                nc.tensor.matmul(vp[:], lhsT=lhs, rhs=vT_t[:], start=True, stop=True)
                nc.vector.tensor_copy(
                    _ap(kvsl_bf, j * 2 * KVW + 2 * D, [[HD + 1, H], [1, HD]]),
                    vp[:].rearrange("p (h x) -> p h x", h=H),
                )
            nc.sync.dma_start(
                out=bass.AP(
                    tensor=kvNodes,
                    offset=s0 * 128 * KVW,
                    ap=[[KVW, 128], [128 * KVW, ns], [1, KVW]],
                ),
                in_=_ap(kvsl[:], 0, [[KVW, ns], [1, KVW]]),
            )


def phase_b(
    nc, tc, t_b, emb_local, kvNodes, out_d,
    iota_bf, ident_bf, qT_t, lnsc_t, lnb_t, eps_t,
    lloc_all, cix_all, embTl,
):
    with (
        tc.tile_pool(name="blk", bufs=3) as blk,
        tc.tile_pool(name="strips", bufs=3) as strips,
        tc.tile_pool(name="small", bufs=3) as small,
        tc.tile_pool(name="fin", bufs=2) as fin,
        tc.tile_pool(name="psT", bufs=2, space="PSUM") as psT,
        tc.tile_pool(name="psS", bufs=2, space="PSUM") as psS,
        tc.tile_pool(name="psQ", bufs=1, space="PSUM") as psQ,
        tc.tile_pool(name="psAcc", bufs=2, space="PSUM") as psAcc,
    ):
        for n in range(NB):
            c0 = n * t_b
            embL = blk.tile([128, 128], F32)
            nc.sync.dma_start(embL[:], emb_local[n * 128 : (n + 1) * 128, :])

            # Q_block = embL @ qT  (f32, straight from preloaded embT_local)
            qp = psQ.tile([128, 128], F32)
            nc.tensor.matmul(
                qp[:],
                lhsT=embTl[:, n * 128 : (n + 1) * 128],
                rhs=qT_t[:],
                start=True,
                stop=True,
            )
            qblk = blk.tile([128, 128], F32)
            nc.scalar.copy(qblk[:], qp[:])

            kvstrip = strips.tile([128, t_b * KVW], F32)
            pstrip = strips.tile([128, t_b * 128], BF16)
            qkstrip = strips.tile([128, t_b * 128], F32)

            for t in range(t_b):
                col = c0 + t
                nc.gpsimd.indirect_dma_start(
                    out=kvstrip[:, t * KVW : (t + 1) * KVW],
                    out_offset=None,
                    in_=kvNodes[:],
                    in_offset=bass.IndirectOffsetOnAxis(
                        ap=cix_all[:, col : col + 1], axis=0
                    ),
                )
                nc.vector.tensor_scalar(
                    out=pstrip[:, t * 128 : (t + 1) * 128],
                    in0=iota_bf[:],
                    scalar1=lloc_all[:, col : col + 1],
                    scalar2=None,
                    op0=mybir.AluOpType.is_equal,
                )
                tp = psT.tile([128, 128], BF16)
                nc.tensor.transpose(
                    out=tp[:],
                    in_=pstrip[:, t * 128 : (t + 1) * 128],
                    identity=ident_bf[:],
                )
                ohT = small.tile([128, 128], F32)
                nc.scalar.copy(ohT[:], tp[:])
                qs = psS.tile([128, 128], F32)
                nc.tensor.matmul(
                    qs[:], lhsT=ohT[:], rhs=qblk[:], start=True, stop=True
                )
                nc.vector.tensor_tensor(
                    out=qkstrip[:, t * 128 : (t + 1) * 128],
                    in0=qs[:],
                    in1=kvstrip[:, t * KVW : t * KVW + D],
                    op=mybir.AluOpType.mult,
                )

            att = small.tile([128, t_b * H], F32)
            nc.vector.tensor_reduce(
                out=att[:].rearrange("p (t h) -> p t h", t=t_b),
                in_=_ap(qkstrip[:], 0, [[128, t_b], [HD, H], [1, HD]]),
                op=mybir.AluOpType.add,
                axis=mybir.AxisListType.X,
            )
            nc.vector.tensor_scalar(
                out=att[:],
                in0=att[:],
                scalar1=10.0,
                scalar2=-10.0,
                op0=mybir.AluOpType.min,
                op1=mybir.AluOpType.max,
            )
            expw = small.tile([128, t_b * H], BF16)
            nc.scalar.activation(
                out=expw[:], in_=att[:], func=mybir.ActivationFunctionType.Exp
            )

            xstrip = strips.tile([128, t_b * XW], BF16)
            nc.vector.tensor_tensor(
                out=_ap(xstrip[:], 0, [[XW, t_b], [HD + 1, H], [1, HD + 1]]),
                in0=_ap(
                    kvstrip[:].bitcast(BF16),
                    2 * D,
                    [[2 * KVW, t_b], [HD + 1, H], [1, HD + 1]],
                ),
                in1=_ap(expw[:], 0, [[H, t_b], [1, H], [0, HD + 1]]),
                op=mybir.AluOpType.mult,
            )

            acc = psAcc.tile([128, XW], F32)
            for t in range(t_b):
                nc.tensor.matmul(
                    acc[:],
                    lhsT=pstrip[:, t * 128 : (t + 1) * 128],
                    rhs=xstrip[:, t * XW : (t + 1) * XW],
                    start=(t == 0),
                    stop=(t == t_b - 1),
                )

            # finalize block n
            accs = fin.tile([128, 136], F32)
            nc.scalar.copy(accs[:], acc[:])
            dinv = fin.tile([128, H], F32)
            nc.vector.tensor_scalar_add(
                dinv[:], _ap(accs[:], HD, [[HD + 1, H]]), 1e-8
            )
            nc.vector.reciprocal(dinv[:], dinv[:])

            res = fin.tile([128, 128], F32)
            nc.vector.tensor_tensor(
                out=res[:].rearrange("p (h x) -> p h x", h=H),
                in0=_ap(accs[:], 0, [[HD + 1, H], [1, HD]]),
                in1=_ap(dinv[:], 0, [[1, H], [0, HD]]),
                op=mybir.AluOpType.mult,
            )
            nc.vector.tensor_add(res[:], res[:], embL[:])

            stats = fin.tile([128, 6], F32)
            nc.vector.bn_stats(out=stats[:], in_=res[:])
            mv = fin.tile([128, 2], F32)
            nc.vector.bn_aggr(out=mv[:], in_=stats[:])

            # 1/sqrt(var+eps) = exp(-0.5*ln(var+eps)) — Ln/Exp/Copy share one
            # activation-function set, so no per-block table reloads.
            lnv = fin.tile([128, 1], F32)
            nc.scalar.activation(
                out=lnv[:],
                in_=mv[:, 1:2],
                func=mybir.ActivationFunctionType.Ln,
                bias=eps_t[:],
                scale=1.0,
            )
            sd = fin.tile([128, 1], F32)
            nc.scalar.activation(
                out=sd[:],
                in_=lnv[:],
                func=mybir.ActivationFunctionType.Exp,
                scale=-0.5,
            )

            xm = fin.tile([128, 128], F32)
            nc.vector.tensor_scalar_sub(xm[:], res[:], mv[:, 0:1])
            y = fin.tile([128, 128], F32)
            nc.vector.scalar_tensor_tensor(
                out=y[:],
                in0=xm[:],
                scalar=sd[:],
                in1=lnsc_t[:],
                op0=mybir.AluOpType.mult,
                op1=mybir.AluOpType.mult,
            )
            nc.vector.tensor_add(y[:], y[:], lnb_t[:])
            nc.sync.dma_start(out_d[n * 128 : (n + 1) * 128, :], y[:])


def _prepare_core_inputs(embeds, edge_index, qTrans, kTrans, vTrans, ln_scale, ln_bias):
    rows = np.asarray(edge_index[0]).astype(np.int64)
    cols = np.asarray(edge_index[1]).astype(np.int64)

    # Balance nodes across ALL 392 blocks by degree (greedy LPT + swap
    # refinement): every per-tile cost scales with t_b = ceil(max block load
    # / 128), so pack blocks to near-uniform edge counts. Nodes may move
    # between cores — the output permutation undoes the layout at the end.
    import heapq

    NBLK = NCORES * NB
    deg = np.bincount(rows, minlength=N).astype(np.int64)
    order_d = np.argsort(-deg, kind="stable")
    loads = np.zeros(NBLK, dtype=np.int64)
    fills = np.zeros(NBLK, dtype=np.int64)
    blk_of = np.empty(N, dtype=np.int64)
    pos_in_blk = np.empty(N, dtype=np.int64)
    heap = [(0, 0, b) for b in range(NBLK)]
    heapq.heapify(heap)
    for i in order_d:
        while True:
            load, fill, b = heapq.heappop(heap)
            if fills[b] < 128:
                break
        blk_of[i] = b
        pos_in_blk[i] = fills[b]
        loads[b] += deg[i]
        fills[b] += 1
        if fills[b] < 128:
            heapq.heappush(heap, (loads[b], fills[b], b))

    target = max(2048, ((E + NBLK * 128 - 1) // (NBLK * 128)) * 128)
    members = [list(np.where(blk_of == b)[0]) for b in range(NBLK)]
    for _ in range(4000):
        bmax = int(loads.argmax())
        if loads[bmax] <= target:
            break
        bmin = int(loads.argmin())
        s_need = int(loads[bmax] - target)
        s_room = int(target - loads[bmin])
        best = None
        dmin_set = {}
        for v in members[bmin]:
            dmin_set.setdefault(int(deg[v]), v)
        for s_try in range(min(s_need, s_room), 0, -1):
            for u in members[bmax]:
                v = dmin_set.get(int(deg[u]) - s_try)
                if v is not None:
                    best = (u, v, s_try)
                    break
            if best:
                break
        if not best:
            break
        u, v, s_try = best
        members[bmax].remove(u)
        members[bmin].remove(v)
        members[bmax].append(v)
        members[bmin].append(u)
        blk_of[u], blk_of[v] = bmin, bmax
        pos_in_blk[u], pos_in_blk[v] = pos_in_blk[v], pos_in_blk[u]
        loads[bmax] -= s_try
        loads[bmin] += s_try

    node_perm = blk_of * 128 + pos_in_blk  # node -> permuted position

    pp = node_perm[rows]  # permuted position of each edge's destination
    order = np.argsort(pp, kind="stable")
    ps = pp[order]
    cs = cols[order]

    core = ps // NBP
    local = ps - core * NBP
    blk = local >> 7
    lloc = (local & 127).astype(np.float32)
    g = core * NB + blk  # global block id, nondecreasing

    counts = np.bincount(g, minlength=NCORES * NB)
    t_b = max(2, int(np.ceil(counts.max() / 128)))
    cap = t_b * 128

    starts = np.zeros(NCORES * NB, dtype=np.int64)
    np.cumsum(counts[:-1], out=starts[1:])
    pos = np.arange(E, dtype=np.int64) - starts[g]
    slot = g * cap + pos

    nslots = NCORES * NB * cap
    lloc_a = np.full(nslots, -1.0, dtype=np.float32)
    cidx_a = np.zeros(nslots, dtype=np.int32)
    lloc_a[slot] = lloc
    cidx_a[slot] = cs.astype(np.int32)

    # [ncores*NB, t_b, 128] -> [ncores, 128(lane), NB*t_b] so per-core index
    # tiles live lane-major in SBUF (one preload DMA each).
    def to_lane_major(a):
        a = a.reshape(NCORES, NB, t_b, 128).transpose(0, 3, 1, 2)
        return np.ascontiguousarray(a.reshape(NCORES, 128, NB * t_b))

    lloc_a = to_lane_major(lloc_a)
    cidx_a = to_lane_major(cidx_a)

    embeds = np.ascontiguousarray(np.asarray(embeds, dtype=np.float32))
    embT_glob = np.zeros((128, NPAD), dtype=np.float32)
    embT_glob[:, :N] = embeds.T
    emb_loc = np.zeros((NCORES * NBP, D), dtype=np.float32)
    emb_loc[node_perm] = embeds
    emb_loc = emb_loc.reshape(NCORES, NBP, D)
    embT_loc = np.ascontiguousarray(emb_loc.transpose(0, 2, 1))

    qT_b = np.ascontiguousarray(np.asarray(qTrans, dtype=np.float32))
    kT_b = np.ascontiguousarray(np.asarray(kTrans, dtype=np.float32))
    vT_b = np.ascontiguousarray(np.asarray(vTrans, dtype=np.float32))
    ln_scale = np.ascontiguousarray(np.asarray(ln_scale, dtype=np.float32))
    ln_bias = np.ascontiguousarray(np.asarray(ln_bias, dtype=np.float32))

    iota = np.broadcast_to(np.arange(128, dtype=np.float32), (128, 128)).astype(BF)
    ident = np.eye(128, dtype=np.float32).astype(BF)

    in_maps = []
    for c in range(NCORES):
        in_maps.append(
            {
                "embT_glob": embT_glob,
                "embT_local": embT_loc[c],
                "emb_local": emb_loc[c],
                "qT": qT_b,
                "kT": kT_b,
                "vT": vT_b,
                "lnsc": ln_scale,
                "lnb": ln_bias,
                "iota": np.ascontiguousarray(iota),
                "ident": ident,
                "lloc": lloc_a[c],
                "cidx": cidx_a[c],
            }
        )
    return in_maps, t_b, node_perm


_PROGRAM_CACHE: dict[int, bass.Bass] = {}


def kernel(embeds, edge_index, qTrans, kTrans, vTrans, ln_scale, ln_bias, **_):
    in_maps, t_b, node_perm = _prepare_core_inputs(
        embeds, edge_index, qTrans, kTrans, vTrans, ln_scale, ln_bias
    )
    nc = _PROGRAM_CACHE.get(t_b)
    if nc is None:
        nc = build_program(t_b)
        _PROGRAM_CACHE[t_b] = nc

    res = run_bass_kernel_spmd(nc, in_maps, core_ids=list(range(NCORES)))
    full = np.concatenate([res.results[c]["out"] for c in range(NCORES)], axis=0)
    return full[node_perm]


if __name__ == "__main__":
    rng = np.random.default_rng(0)
    inputs = {
        "embeds": rng.standard_normal((N, D), dtype=np.float32),
        "edge_index": rng.integers(0, N, size=(2, E)).astype(np.int64),
        "qTrans": (rng.standard_normal((D, D), dtype=np.float32) / np.sqrt(D)).astype(
            np.float32
        ),
        "kTrans": (rng.standard_normal((D, D), dtype=np.float32) / np.sqrt(D)).astype(
            np.float32
        ),
        "vTrans": (rng.standard_normal((D, D), dtype=np.float32) / np.sqrt(D)).astype(
            np.float32
        ),
        "ln_scale": np.ones(D, dtype=np.float32),
        "ln_bias": np.zeros(D, dtype=np.float32),
    }
    out = kernel(**inputs)
    print("kernel output", out.shape, out.dtype, np.isfinite(out).all())
